# revision 1
# baseline (speedup 1.0000x reference)
"""GAT (Cora-style) forward pass distributed across 8 NeuronCores.

Sharding: head-parallel. H=8 attention heads, one per core. Each core
holds its head's W [512,64], a_src/a_dst [64], plus the full x [4096,512]
and adj [4096,4096] (replicated), computes its [N,N] score matrix,
masked softmax, aggregation, and ELU locally. The per-head [N,64]
outputs are gathered on host and concatenated to [N, 512].
"""

import numpy as np
import jax
import jax.numpy as jnp
from functools import partial

N = 4096
F_IN = 512
H = 8
D = 64
SLOPE = 0.2
NEG_INF = -9e15

_compiled = None


def _per_head(x, adj_f, W_h, a_src_h, a_dst_h):
    # x: [N, F_IN], adj_f: [N, N] float32 (1.0 edge / 0.0 no edge)
    # W_h: [F_IN, D], a_src_h/a_dst_h: [D]
    Wh = x @ W_h                                   # [N, D]
    s = Wh @ a_src_h                               # [N]
    t = Wh @ a_dst_h                               # [N]
    e = s[:, None] + t[None, :]                    # [N, N]
    e = jnp.where(e >= 0, e, SLOPE * e)            # LeakyReLU
    e = jnp.where(adj_f > 0, e, NEG_INF)           # mask non-edges
    m = jnp.max(e, axis=-1, keepdims=True)
    p = jnp.exp(e - m)
    att = p / jnp.sum(p, axis=-1, keepdims=True)   # masked softmax
    out = att @ Wh                                 # [N, D]
    return jnp.where(out > 0, out, jnp.expm1(out))  # ELU


def _get_compiled():
    global _compiled
    if _compiled is None:
        devs = jax.devices()[:H]
        _compiled = jax.pmap(
            _per_head,
            in_axes=(None, None, 0, 0, 0),
            devices=devs,
        )
    return _compiled


def kernel(x, adj, W, a_src, a_dst):
    x = np.asarray(x, dtype=np.float32)
    adj_f = np.asarray(adj, dtype=np.float32)
    W = np.asarray(W, dtype=np.float32)
    a_src = np.asarray(a_src, dtype=np.float32)
    a_dst = np.asarray(a_dst, dtype=np.float32)

    fn = _get_compiled()
    out = fn(x, adj_f, W, a_src, a_dst)            # [H, N, D], head-sharded
    out = np.asarray(out)
    return np.transpose(out, (1, 0, 2)).reshape(N, H * D).astype(np.float32)
